# revision 1
# baseline (speedup 1.0000x reference)
"""2D DWT (db4, circular pad, stride-2) forward on 8 Trainium2 NeuronCores.

Strategy (pure data parallel, 12 images of 512x512 per core):
Both separable filter passes are expressed as banded fp32 matmuls on the
TensorEngine, so no transposes are needed anywhere:

  stage 1 (filter along H):  V[w, (hj,a)]   = sum_h  X[h, w] * M[h, (hj,a)]
  stage 2 (filter along W):  out[hj,(wj,b)] = sum_w  V[w, a*256+hj] * M[w, (wj,b)]

M is the 512x512 interleaved filter-bank matrix M[i, 2j+f] = dec[f][(i-2j)%512]
(8 nonzeros per column). Each 128-row chunk of M only has ~67 nonzero j
columns, so each PSUM accumulation streams just the banded column slices
(~536 of 2048 columns per bank) instead of dense 512-wide matmuls — this
keeps fp32 (4 cycles/row) near the HBM roofline. PSUM's per-element
has_written bit handles the overlapping column ranges across K-chunks.
"""

import sys

sys.path.insert(0, "/opt/trn_rl_repo")

import numpy as np

L = 512
NJ = L // 2  # 256
TAPS = 8
N_CORES = 8
IMGS_PER_CORE = 12  # 32 batch * 3 channels / 8 cores

_compiled = {}


def _build_M(dec: np.ndarray) -> np.ndarray:
    """M[i, 2*j + f] = dec[f][(i - 2j) mod 512]; filters interleaved so each
    128-row chunk's nonzero columns form one contiguous range (plus wrap)."""
    M = np.zeros((L, L), dtype=np.float32)
    i = np.arange(L)[:, None]
    j = np.arange(NJ)[None, :]
    k = (i - 2 * j) % L
    mask = k < TAPS
    for f in range(2):
        M[:, f::2] = np.where(mask, np.asarray(dec[f])[np.minimum(k, TAPS - 1)], 0.0)
    return M


def _col_slices(c: int):
    """Interleaved nonzero column ranges of M rows [128c, 128c+128):
    j in [64c-3, 64c+63] (mod 256) -> interleaved cols [2j, 2j+1]."""
    lo_j, hi_j = 64 * c - 3, 64 * c + 63
    if lo_j < 0:
        return [(0, 2 * (hi_j + 1)), (2 * (lo_j % NJ), 2 * NJ)]
    return [(2 * lo_j, 2 * (hi_j + 1))]


def _build_nc():
    import concourse.bass as bass  # noqa: F401
    import concourse.tile as tile
    from concourse import bacc, mybir

    f32 = mybir.dt.float32
    nc = bacc.Bacc("TRN2", target_bir_lowering=False, debug=False,
                   num_devices=N_CORES)
    x_d = nc.dram_tensor("x", [IMGS_PER_CORE, L, L], f32, kind="ExternalInput")
    m_d = nc.dram_tensor("m", [L, L], f32, kind="ExternalInput")
    o_d = nc.dram_tensor("out", [IMGS_PER_CORE, 4, NJ, NJ], f32,
                         kind="ExternalOutput")

    with tile.TileContext(nc) as tc:
        with (
            tc.tile_pool(name="mpool", bufs=1) as mpool,
            tc.tile_pool(name="xpool", bufs=3) as xpool,
            tc.tile_pool(name="vpool", bufs=2) as vpool,
            tc.tile_pool(name="opool", bufs=4) as opool,
            tc.tile_pool(name="pvpool", bufs=4, space="PSUM") as pvpool,
            tc.tile_pool(name="popool", bufs=4, space="PSUM") as popool,
        ):
            # M: 4 h-chunks side by side -> (128, 4*512)
            mt = mpool.tile([128, 4 * L], f32, tag="mt")
            nc.sync.dma_start(
                mt[:].rearrange("p (c w) -> p c w", c=4),
                m_d[:].rearrange("(c p) w -> p c w", p=128),
            )

            for img in range(IMGS_PER_CORE):
                # image: 4 h-chunks side by side -> (128, 4*512), free = w
                xt = xpool.tile([128, 4 * L], f32, tag="xt")
                nc.sync.dma_start(
                    xt[:].rearrange("p (c w) -> p c w", c=4),
                    x_d[img].rearrange("(c p) w -> p c w", p=128),
                )

                # stage 1: V[w, (hj,a)], w-chunk wc in vt cols [512wc, 512wc+512),
                # de-interleaved: [0:256) = a=0 (lo), [256:512) = a=1 (hi)
                vt = vpool.tile([128, 4 * L], f32, tag="vt")
                for wc in range(4):
                    pv = pvpool.tile([128, L], f32, tag="pv")
                    mms = [
                        (hc, c0, c1)
                        for hc in range(4)
                        for (c0, c1) in _col_slices(hc)
                    ]
                    for n, (hc, c0, c1) in enumerate(mms):
                        nc.tensor.matmul(
                            pv[:, c0:c1],
                            xt[:, L * hc + 128 * wc : L * hc + 128 * wc + 128],
                            mt[:, L * hc + c0 : L * hc + c1],
                            start=(n == 0),
                            stop=(n == len(mms) - 1),
                        )
                    nc.vector.tensor_copy(vt[:, L * wc : L * wc + NJ], pv[:, 0:L:2])
                    nc.vector.tensor_copy(
                        vt[:, L * wc + NJ : L * wc + L], pv[:, 1:L:2]
                    )

                # stage 2: per (a, hjc) one PSUM bank of out[hj, (wj,b)]
                # subband s = a + 2b; ot per hjc: (128, 4*256), free = (s, wj)
                ots = []
                for hjc in range(2):
                    ot = opool.tile([128, 4 * NJ], f32, tag="ot")
                    ots.append(ot)
                    for a in range(2):
                        po = popool.tile([128, L], f32, tag="po")
                        mms = [
                            (wc, c0, c1)
                            for wc in range(4)
                            for (c0, c1) in _col_slices(wc)
                        ]
                        for n, (wc, c0, c1) in enumerate(mms):
                            nc.tensor.matmul(
                                po[:, c0:c1],
                                vt[:, L * wc + NJ * a + 128 * hjc :
                                   L * wc + NJ * a + 128 * hjc + 128],
                                mt[:, L * wc + c0 : L * wc + c1],
                                start=(n == 0),
                                stop=(n == len(mms) - 1),
                            )
                        # b=0 (cols 0::2) -> subband a; b=1 (cols 1::2) -> 2+a
                        nc.scalar.copy(
                            ot[:, NJ * a : NJ * a + NJ], po[:, 0:L:2]
                        )
                        nc.scalar.copy(
                            ot[:, NJ * (2 + a) : NJ * (2 + a) + NJ], po[:, 1:L:2]
                        )
                for hjc in range(2):
                    nc.sync.dma_start(
                        o_d[img, :, 128 * hjc : 128 * hjc + 128, :].rearrange(
                            "s p w -> p s w"
                        ),
                        ots[hjc][:].rearrange("p (s w) -> p s w", s=4),
                    )

    nc.finalize()
    return nc


def kernel(x: np.ndarray, dec: np.ndarray) -> np.ndarray:
    from concourse.bass_utils import run_bass_kernel_spmd

    x = np.ascontiguousarray(np.asarray(x, dtype=np.float32))
    dec = np.asarray(dec, dtype=np.float32)
    B, C, H, W = x.shape
    assert (B, C, H, W) == (32, 3, 512, 512) and dec.shape == (2, 8)

    if "nc" not in _compiled:
        _compiled["nc"] = _build_nc()
    nc = _compiled["nc"]

    M = _build_M(dec)
    x96 = x.reshape(B * C, H, W)
    in_maps = [
        {"x": x96[IMGS_PER_CORE * c : IMGS_PER_CORE * (c + 1)], "m": M}
        for c in range(N_CORES)
    ]
    res = run_bass_kernel_spmd(nc, in_maps, list(range(N_CORES))).results
    out = np.concatenate([r["out"] for r in res], axis=0)  # (96, 4, 256, 256)
    return out.reshape(B, C * 4, H // 2, W // 2)
